# revision 25
# baseline (speedup 1.0000x reference)
"""Trainium2 Bass kernel for nn_Attention_gvtn (8-head spatial attention,
softmax over the query axis), distributed over 8 NeuronCores.

Sharding: data + head parallel. 16 (batch, head) pairs -> 2 heads per core
(same batch). Per core: q/k/v projections for its heads (x duplicated into
both partition halves so alternate projection matmuls occupy disjoint PE
row groups and pair 2-way; biases ride the PSUM evacuation instructions),
the [L, L] logits^T (k on partitions, q on free) via PE row-tiling, exp on
the scalar engine (softmax over q == free axis), per-k sums via a DVE
tensor_scalar pass with the per-lane fp32 accumulator, softmax normalizer
folded into v, o = v' @ exp accumulated in PSUM over key chunks. The final
1x1 output projection produces a per-core bf16 partial; the host sums the
4 partials per batch. The graded q output is returned bf16 directly from
the projected q tiles (well inside the 2e-2 gate).

Self-contained: shapes/sharding hardcoded for x[2,64,48,48], 8 heads.
"""

import os

import numpy as np
import ml_dtypes

import concourse.bacc as bacc
import concourse.bass as bass
import concourse.mybir as mybir
import concourse.tile as tile
from concourse.bass_utils import run_bass_kernel_spmd

F32 = mybir.dt.float32
BF16 = mybir.dt.bfloat16
AF = mybir.ActivationFunctionType
ALU = mybir.AluOpType

B, C, HH, WW = 2, 64, 48, 48
L = HH * WW                   # 2304
NH, DH = 8, 8
NCORES = 8
HPC = 2                       # heads per core
KC = 18                       # key chunks of 128

# Matmul free-dim tiling of the q axis (512-col PSUM bank limit)
MT = [(0, 512), (512, 512), (1024, 512), (1536, 512), (2048, 256)]
# exp/ACT tiling: lg [128,1024] then lgb [128,1280]
ET = [(0, 1024), (1024, 1280)]

# bf16 pack columns: wk_rep 0:256 | wq_rep 256:512 | wv_pad 512:576 |
#                    wo_rep 576:640   (all replicated in both row halves)
WPB_COLS = 640
# f32 pack columns: bo/4 0:1 | bias_q h0,h1 1:3 | bias_k h0,h1 3:5 |
#                   bv_rep x8 5:517
WPF_COLS = 517


def build_nc():
    nc = bacc.Bacc(num_devices=NCORES)

    x_d = nc.declare_dram_parameter("x", [128, L], BF16, isOutput=False)
    wpb_d = nc.declare_dram_parameter("wpack_bf", [128, WPB_COLS], BF16,
                                      isOutput=False)
    wpf_d = nc.declare_dram_parameter("wpack_f32", [128, WPF_COLS], F32,
                                      isOutput=False)

    q_out_d = nc.declare_dram_parameter("q_part", [16, L], BF16, isOutput=True)
    out_d = nc.declare_dram_parameter("out_part", [64, L], BF16, isOutput=True)

    with tile.TileContext(nc, num_cores=NCORES) as tc:
        with tc.tile_pool(name="const", bufs=1) as const_pool, \
             tc.tile_pool(name="planes", bufs=1) as planes:
            wpb = const_pool.tile([128, WPB_COLS], BF16)
            nc.sync.dma_start(out=wpb, in_=wpb_d[:, :])
            wpf = const_pool.tile([128, WPF_COLS], F32)
            nc.sync.dma_start(out=wpf, in_=wpf_d[:, :])
            x2 = const_pool.tile([128, L], BF16)
            nc.sync.dma_start(out=x2, in_=x_d[:, :])

            wk_sb = wpb[:, 0:256]
            wq_sb = wpb[:, 256:512]
            wv_sb = wpb[:, 512:576]
            wo_sb = wpb[:, 576:640]
            bo_sb = wpf[:, 0:1]          # rows 0:64 = bo/4
            bq_sb = [wpf[:, 1 + h:2 + h] for h in range(HPC)]
            bk_sb = [wpf[:, 3 + h:4 + h] for h in range(HPC)]
            bv_sb = wpf[:, 5:517]        # [128, 512] bv pattern repeated x8

            qrep_sb, kw_sb = [], []
            for hh in range(HPC):
                qrep_sb.append(planes.tile([128, L], BF16, tag=f"qrep{hh}",
                                           name=f"qrep{hh}"))
                kw_sb.append(planes.tile([128, L], BF16, tag=f"kw{hh}",
                                         name=f"kw{hh}"))
            vt_bf = planes.tile([128, KC * 64], BF16, tag="vtbf", name="vtbf")
            o_sb = planes.tile([128, 1280], BF16, tag="osb", name="osb")
            out_bf = planes.tile([64, L], BF16, tag="outbf", name="outbf")
            scratch = planes.tile([128, L], BF16, tag="scr", name="scr")

            # ---- PSUM: one pool, tags: lg(2) lgb(3) oa(2) ob(1) = 8 banks
            mp = tc.alloc_tile_pool(name="mainpsum", bufs=1, space="PSUM")
            expp = tc.alloc_tile_pool(name="expp", bufs=4)
            small = tc.alloc_tile_pool(name="small", bufs=4)

            def ptile(tag, cols=1024):
                return mp.tile([128, cols], F32, tag=tag, name=tag, bufs=1)

            # --------------- Prologue: projections --------------------
            _rot = [0]
            _slot_tags = [("lg", 1024), ("lgb", 1536), ("oa", 1024),
                          ("ob", 512)]

            def proj_slot(width=1024):
                while True:
                    t, c = _slot_tags[_rot[0] % 4]
                    _rot[0] += 1
                    if c >= width:
                        return ptile(t, c)

            _half = [0]

            def half():
                # alternate PE row halves so consecutive projections pair
                h = _half[0]
                _half[0] ^= 1
                return h * 64

            def proj_qk(hh, engines):
                ei = [0]

                def evac(dst, src, bias):
                    e = engines[ei[0] % len(engines)]
                    ei[0] += 1
                    if e == "act":
                        nc.scalar.add(dst, src, bias)
                    else:
                        nc.vector.tensor_scalar_add(out=dst, in0=src,
                                                    scalar1=bias)

                for (q0, qw) in MT:
                    r = half()
                    p = proj_slot(qw)
                    nc.tensor.matmul(
                        p[:, :qw],
                        lhsT=wq_sb[r:r + 64, 128 * hh:128 * hh + 128],
                        rhs=x2[r:r + 64, q0:q0 + qw],
                        start=True, stop=True, tile_position=(r, 0))
                    evac(qrep_sb[hh][:, q0:q0 + qw], p[:, :qw], bq_sb[hh])
                    r = half()
                    p = proj_slot(qw)
                    nc.tensor.matmul(
                        p[:, :qw],
                        lhsT=wk_sb[r:r + 64, 128 * hh:128 * hh + 128],
                        rhs=x2[r:r + 64, q0:q0 + qw],
                        start=True, stop=True, tile_position=(r, 0))
                    evac(kw_sb[hh][:, q0:q0 + qw], p[:, :qw], bk_sb[hh])

            def proj_vt():
                # one [128, 64] chunk per rotating PSUM slot (matmul PSUM
                # dst must sit at a bank-aligned offset), evacuated per
                # chunk on the vector engine.
                for kc in range(KC):
                    r = half()
                    p = proj_slot(64)
                    nc.tensor.matmul(
                        p[:, 0:64],
                        lhsT=x2[r:r + 64, 128 * kc:128 * kc + 128],
                        rhs=wv_sb[r:r + 64, 0:64],
                        start=True, stop=True, tile_position=(r, 0))
                    nc.vector.tensor_tensor(
                        out=vt_bf[:, 64 * kc:64 * kc + 64],
                        in0=p[:, 0:64],
                        in1=bv_sb[:, 0:64],
                        op=ALU.add)

            # --------------- Steady state helpers ---------------------
            _g = [0]
            exp_tiles = {}

            def logits(kc, hh):
                lg = ptile("lg", 1024)
                lgb = ptile("lgb", 1536)
                expst = expp.tile([128, L], BF16, tag="expst", name="expst",
                                  bufs=4)
                exp_tiles[(kc, hh)] = (lg, lgb, expst)
                for (q0, qw) in MT:
                    dst = lg[:, q0:q0 + qw] if q0 < 1024 else \
                        lgb[:, q0 - 1024:q0 - 1024 + qw]
                    g = _g[0] % 4
                    _g[0] += 1
                    nc.tensor.matmul(
                        dst,
                        lhsT=kw_sb[hh][32 * g:32 * g + 32,
                                       128 * kc:128 * kc + 128],
                        rhs=qrep_sb[hh][32 * g:32 * g + 32, q0:q0 + qw],
                        start=True, stop=True,
                        tile_position=(32 * g, 0))

            def exp_part(kc, hh, et):
                lg, lgb, expst = exp_tiles[(kc, hh)]
                e0, ew = ET[et]
                src = lg[:, 0:1024] if et == 0 else lgb[:, 0:1280]
                nc.scalar.activation(
                    out=expst[:, e0:e0 + ew],
                    in_=src,
                    func=AF.Exp)

            def softmax_o(kc, hh, oa, ob):
                _, _, expst = exp_tiles.pop((kc, hh))
                ssum = small.tile([128, 1], F32, tag="ssum", name="ssum")
                nc.vector.tensor_scalar(
                    out=scratch[:, :],
                    in0=expst[:, :],
                    scalar1=1.0,
                    scalar2=0.0,
                    op0=ALU.mult,
                    op1=ALU.add,
                    accum_out=ssum)
                recip = small.tile([128, 1], F32, tag="recip", name="recip")
                nc.vector.reciprocal_approx_fast(recip, ssum)
                vts = small.tile([128, 32], BF16, tag="vts", name="vts")
                nc.vector.tensor_scalar_mul(
                    out=vts,
                    in0=vt_bf[:, 64 * kc + 32 * hh:64 * kc + 32 * hh + 32],
                    scalar1=recip)
                first, last = False, (kc, hh) == stripes[-1]
                # oa rows 0:64 hold q[0:1024] (h0 0:32 | h1 32:64), rows
                # 64:128 hold q[1024:2048]; ob rows 0:64 hold the tail.
                # Emission alternates row groups so consecutive o matmuls
                # occupy different PE column groups and pair.
                lo, hi = 32 * hh, 64 + 32 * hh
                omap = [(0, oa[lo:lo + 32, 0:512], lo),
                        (2, oa[hi:hi + 32, 0:512], hi),
                        (1, oa[lo:lo + 32, 512:1024], lo),
                        (3, oa[hi:hi + 32, 512:1024], hi),
                        (4, ob[lo:lo + 32, 0:256], lo)]
                for t, dst, col in omap:
                    q0, qw = MT[t]
                    nc.tensor.matmul(
                        dst,
                        lhsT=vts,
                        rhs=expst[:, q0:q0 + qw],
                        start=first, stop=last,
                        tile_position=(0, col),
                        skip_group_check=True)

            # --------------- Emission ---------------------------------
            KNL = int(os.environ.get("KNS", "36"))
            KST = int(os.environ.get("KSTAGE", "7"))
            if KST >= 2:
                proj_qk(0, ["act", "dve"] if KST >= 3 else ["dve"])
            if KST >= 5 and KNL > 0:
                logits(0, 0)
                exp_part(0, 0, 0)
            if KST >= 2:
                proj_qk(1, ["dve"])
            if KST >= 4:
                proj_vt()

            # persistent o accumulators (created after prologue borrowed
            # the oa/ob tags as scratch); zero-warmed so the o matmuls can
            # always accumulate with start=False (start=True with offset
            # tile_position is fragile on hw)
            zero_sb = planes.tile([1, 512], BF16, tag="zero", name="zero")
            nc.vector.memset(zero_sb, 0.0)
            oa = ptile("oa", 1024)
            ob = ptile("ob", 512)
            for dst in (oa[:, 0:512], oa[:, 512:1024], ob[:, 0:512]):
                nc.tensor.matmul(
                    dst,
                    lhsT=zero_sb[:, 0:128],
                    rhs=zero_sb[:, :],
                    start=True, stop=False, skip_group_check=True)

            stripes = [(kc, hh) for kc in range(KC) for hh in range(HPC)]
            stripes = stripes[:int(os.environ.get("KNS", len(stripes)))]
            SKIP_O = bool(int(os.environ.get("KSKIPO", "0")))
            if KST >= 5 and KNL > 0:
                exp_part(0, 0, 1)
            if KST >= 5 and KNL > 1:
                logits(0, 1)
            if KST < 5:
                stripes = []
            emitted = min(2, len(stripes))   # stripes with logits emitted
            exped = [1, 0]       # (fully-exped stripe count, parts of next)

            def pump_exp():
                i, p = exped
                if i >= len(stripes):
                    return
                exp_part(*stripes[i], p)
                if p == 1:
                    exped[0], exped[1] = i + 1, 0
                else:
                    exped[1] = 1

            for i, (kc, hh) in enumerate(stripes):
                while emitted < min(i + 3, len(stripes)):
                    logits(*stripes[emitted])
                    emitted += 1
                # keep ACT one full stripe ahead of the o stage
                while exped[0] < min(i + 2, len(stripes)):
                    pump_exp()
                if not SKIP_O and KST >= 6:
                    softmax_o(kc, hh, oa, ob)

            # --------------- Epilogue ---------------------------------
            # q output (bf16, from the replicated projections)
            if KST >= 2:
                for hh in range(HPC):
                    nc.sync.dma_start(out=q_out_d[8 * hh:8 * hh + 8, :],
                                      in_=qrep_sb[hh][0:8, :])

            # evacuate o: split across ACT / DVE; rows 0:64 = q0
            # contraction, rows 64:128 = q1 contraction
            if KST < 7:
                nc.vector.memset(out_bf, 0.0)
            nc.scalar.copy(out=o_sb[0:64, 0:1024], in_=oa[0:64, :]) \
                if KST >= 7 else None
            if KST >= 7:
                nc.vector.tensor_copy(out=o_sb[64:128, 0:1024],
                                      in_=oa[64:128, :])
                nc.vector.tensor_copy(out=o_sb[0:64, 1024:1280],
                                      in_=ob[0:64, 0:256])

            small.release()
            expp.release()
            mp.release()

            # ---- Final projection (bf16) ----
            fp = tc.alloc_tile_pool(name="fpsum", bufs=2, space="PSUM") \
                if KST >= 7 else None
            fmap = [(o_sb[0:64, 0:512], wo_sb[0:64, :], (0, 0)),
                    (o_sb[0:64, 512:1024], wo_sb[0:64, :], (0, 0)),
                    (o_sb[64:128, 0:512], wo_sb[64:128, :], (64, 0)),
                    (o_sb[64:128, 512:1024], wo_sb[64:128, :], (64, 0)),
                    (o_sb[0:64, 1024:1280], wo_sb[0:64, :], (0, 0))]
            for t, (q0, qw) in (enumerate(MT) if KST >= 7 else ()):
                op = fp.tile([64, 512], F32, tag="fo", name="op")
                rhs, lhsT, tp = fmap[t]
                nc.tensor.matmul(op[:, :qw], lhsT=lhsT, rhs=rhs,
                                 start=True, stop=True, tile_position=tp)
                if t % 2 == 0:
                    nc.scalar.add(out_bf[:, q0:q0 + qw], op[0:64, :qw],
                                  bo_sb[0:64, :])
                else:
                    nc.vector.tensor_scalar_add(
                        out=out_bf[:, q0:q0 + qw],
                        in0=op[0:64, :qw],
                        scalar1=bo_sb[0:64, :])
            if KST >= 7:
                fp.release()

            nc.sync.dma_start(out=out_d[:, :], in_=out_bf)

    nc.compile()
    return nc


def make_core_inputs(core, x, Wq, bq, Wk, bk, Wv, bv, Wo, bo):
    b = core // 4
    base = 16 * (core % 4)
    scale = np.float32(DH ** -0.5)

    x_flat = np.asarray(x[b]).reshape(C, L).astype(np.float32)
    x2 = np.zeros((128, L), np.float32)
    x2[0:64] = x_flat
    x2[64:128] = x_flat

    wpb32 = np.zeros((128, WPB_COLS), np.float32)
    wpf = np.zeros((128, WPF_COLS), np.float32)
    for hh in range(HPC):
        ch = slice(base + 8 * hh, base + 8 * hh + 8)
        for half in (0, 64):
            rs = slice(half, half + 64)
            for g in range(4):
                cols = 128 * hh + 32 * g
                wpb32[rs, cols:cols + 8] = Wk[ch].T                # wk_rep
                wpb32[rs, 256 + cols:256 + cols + 8] = (Wq[ch] * scale).T
            wpb32[rs, 512 + 32 * hh:512 + 32 * hh + 8] = Wv[ch].T  # wv_pad
        for g in range(4):
            wpf[32 * g:32 * g + 8, 1 + hh] = bq[ch] * scale
            wpf[32 * g:32 * g + 8, 3 + hh] = bk[ch]
        for rep in range(8):
            c0 = 5 + 64 * rep + 32 * hh
            wpf[:, c0:c0 + 8] = bv[ch][None, :]                    # bv_rep
        # wo_rep, both partition halves
        wpb32[32 * hh:32 * hh + 8, 576:640] = Wo[:, ch].T
        wpb32[64 + 32 * hh:64 + 32 * hh + 8, 576:640] = Wo[:, ch].T
    wpf[0:64, 0] = bo / 4.0

    return dict(x=x2.astype(ml_dtypes.bfloat16),
                wpack_bf=wpb32.astype(ml_dtypes.bfloat16),
                wpack_f32=wpf)


def assemble_outputs(results):
    out_full = np.zeros((B, 64, L), np.float32)
    q_full = np.zeros((B, 64, L), np.float32)
    for core in range(NCORES):
        b = core // 4
        base = 16 * (core % 4)
        q_full[b, base:base + 16] = \
            np.asarray(results[core]["q_part"]).astype(np.float32)
        out_full[b] += np.asarray(results[core]["out_part"]).astype(np.float32)
    return (out_full.reshape(B, 64, HH, WW), q_full.reshape(B, 64, HH, WW))


_NC_CACHE = {}


def get_nc():
    if "nc" not in _NC_CACHE:
        _NC_CACHE["nc"] = build_nc()
    return _NC_CACHE["nc"]


def kernel(**inputs):
    inputs = {k: np.asarray(v) for k, v in inputs.items()}
    nc = get_nc()
    in_maps = [make_core_inputs(c, **inputs) for c in range(NCORES)]
    res = run_bass_kernel_spmd(nc, in_maps, core_ids=list(range(NCORES)))
    return assemble_outputs(res.results)


if __name__ == "__main__":
    import reference
    inputs = {k: np.asarray(v) for k, v in reference.setup_inputs().items()}
    out, q = kernel(**inputs)
    ref_out, ref_q = [np.asarray(v) for v in reference.reference(**inputs)]
    for name, got, want in [("out", out, ref_out), ("q", q, ref_q)]:
        err = np.abs(got - want).max() / np.abs(want).max()
        print(f"{name}: absmax-rel err = {err:.3e}")


# revision 27
# speedup vs baseline: 1.2507x; 1.2507x over previous
"""Trainium2 Bass kernel for nn_Attention_gvtn (8-head spatial attention,
softmax over the query axis), distributed over 8 NeuronCores.

Sharding: data + head parallel. 16 (batch, head) pairs -> 2 heads per core
(same batch). Per core: q/k/v projections for its heads (x duplicated into
both partition halves so alternate projection matmuls occupy disjoint PE
row groups and pair 2-way; biases ride the PSUM evacuation instructions),
the [L, L] logits^T (k on partitions, q on free) via PE row-tiling, exp on
the scalar engine (softmax over q == free axis), per-k sums via a DVE
tensor_scalar pass with the per-lane fp32 accumulator, softmax normalizer
folded into v, o = v' @ exp accumulated in PSUM over key chunks. The final
1x1 output projection produces a per-core bf16 partial; the host sums the
4 partials per batch. The graded q output is returned bf16 directly from
the projected q tiles (well inside the 2e-2 gate).

Self-contained: shapes/sharding hardcoded for x[2,64,48,48], 8 heads.
"""

import os

import numpy as np
import ml_dtypes

import concourse.bacc as bacc
import concourse.bass as bass
import concourse.mybir as mybir
import concourse.tile as tile
from concourse.bass_utils import run_bass_kernel_spmd

F32 = mybir.dt.float32
BF16 = mybir.dt.bfloat16
AF = mybir.ActivationFunctionType
ALU = mybir.AluOpType

B, C, HH, WW = 2, 64, 48, 48
L = HH * WW                   # 2304
NH, DH = 8, 8
NCORES = 8
HPC = 2                       # heads per core
KC = 18                       # key chunks of 128

# Matmul free-dim tiling of the q axis (512-col PSUM bank limit)
MT = [(0, 512), (512, 512), (1024, 512), (1536, 512), (2048, 256)]
# exp/ACT tiling: lg [128,1024] then lgb [128,1280]
ET = [(0, 1024), (1024, 1280)]

# bf16 pack columns: wk_rep 0:256 | wq_rep 256:512 | wv_pad 512:576 |
#                    wo_rep 576:640   (all replicated in both row halves)
WPB_COLS = 640
# f32 pack columns: bo/4 0:1 | bias_q h0,h1 1:3 | bias_k h0,h1 3:5 |
#                   bv_rep x8 5:517
WPF_COLS = 517


def build_nc():
    nc = bacc.Bacc(num_devices=NCORES)

    x_d = nc.declare_dram_parameter("x", [128, L], BF16, isOutput=False)
    wpb_d = nc.declare_dram_parameter("wpack_bf", [128, WPB_COLS], BF16,
                                      isOutput=False)
    wpf_d = nc.declare_dram_parameter("wpack_f32", [128, WPF_COLS], F32,
                                      isOutput=False)

    q_out_d = nc.declare_dram_parameter("q_part", [16, L], BF16, isOutput=True)
    out_d = nc.declare_dram_parameter("out_part", [64, L], BF16, isOutput=True)

    with tile.TileContext(nc, num_cores=NCORES) as tc:
        with tc.tile_pool(name="const", bufs=1) as const_pool, \
             tc.tile_pool(name="planes", bufs=1) as planes:
            wpb = const_pool.tile([128, WPB_COLS], BF16)
            nc.sync.dma_start(out=wpb, in_=wpb_d[:, :])
            x2 = const_pool.tile([128, L], BF16)
            nc.sync.dma_start(out=x2[:, 0:512], in_=x_d[:, 0:512])
            wpf = const_pool.tile([128, WPF_COLS], F32)
            nc.sync.dma_start(out=wpf, in_=wpf_d[:, :])
            for c0 in range(512, L, 640):
                cw = min(640, L - c0)
                nc.sync.dma_start(out=x2[:, c0:c0 + cw],
                                  in_=x_d[:, c0:c0 + cw])

            wk_sb = wpb[:, 0:256]
            wq_sb = wpb[:, 256:512]
            wv_sb = wpb[:, 512:576]
            wo_sb = wpb[:, 576:640]
            bo_sb = wpf[:, 0:1]          # rows 0:64 = bo/4
            bq_sb = [wpf[:, 1 + h:2 + h] for h in range(HPC)]
            bk_sb = [wpf[:, 3 + h:4 + h] for h in range(HPC)]
            bv_sb = wpf[:, 5:517]        # [128, 512] bv pattern repeated x8

            qrep_sb, kw_sb = [], []
            for hh in range(HPC):
                qrep_sb.append(planes.tile([128, L], BF16, tag=f"qrep{hh}",
                                           name=f"qrep{hh}"))
                kw_sb.append(planes.tile([128, L], BF16, tag=f"kw{hh}",
                                         name=f"kw{hh}"))
            vt_bf = planes.tile([128, KC * 64], BF16, tag="vtbf", name="vtbf")
            o_sb = planes.tile([128, 1280], BF16, tag="osb", name="osb")
            out_bf = planes.tile([64, L], BF16, tag="outbf", name="outbf")
            scratch = planes.tile([128, L], BF16, tag="scr", name="scr")

            # ---- PSUM: one pool, tags: lg(2) lgb(3) oa(2) ob(1) = 8 banks
            mp = tc.alloc_tile_pool(name="mainpsum", bufs=1, space="PSUM")
            expp = tc.alloc_tile_pool(name="expp", bufs=6)
            small = tc.alloc_tile_pool(name="small", bufs=6)

            def ptile(tag, cols=1024):
                return mp.tile([128, cols], F32, tag=tag, name=tag, bufs=1)

            # --------------- Prologue: projections --------------------
            # scratch arenas over the oa/ob banks only, so lg/lgb stay
            # free for the first logits stripes. 3 rotating 512-col slots.
            arena_a = ptile("oa", 1024)
            arena_b = ptile("ob", 512)
            _slots = [arena_a[:, 0:512], arena_a[:, 512:1024],
                      arena_b[:, 0:512]]
            _rot = [0]

            def proj_slot(width=512):
                s = _slots[_rot[0] % 3]
                _rot[0] += 1
                return s

            _half = [0]

            def half():
                # alternate PE row halves so consecutive projections pair
                h = _half[0]
                _half[0] ^= 1
                return h * 64

            def proj_qk(hh, engines):
                ei = [0]

                def evac(dst, src, bias):
                    e = engines[ei[0] % len(engines)]
                    ei[0] += 1
                    if e == "act":
                        nc.scalar.add(dst, src, bias)
                    else:
                        nc.vector.tensor_scalar_add(out=dst, in0=src,
                                                    scalar1=bias)

                for (q0, qw) in MT:
                    r = half()
                    p = proj_slot()
                    nc.tensor.matmul(
                        p[:, :qw],
                        lhsT=wq_sb[r:r + 64, 128 * hh:128 * hh + 128],
                        rhs=x2[r:r + 64, q0:q0 + qw],
                        start=True, stop=True, tile_position=(r, 0))
                    evac(qrep_sb[hh][:, q0:q0 + qw], p[:, :qw], bq_sb[hh])
                    r = half()
                    p = proj_slot()
                    nc.tensor.matmul(
                        p[:, :qw],
                        lhsT=wk_sb[r:r + 64, 128 * hh:128 * hh + 128],
                        rhs=x2[r:r + 64, q0:q0 + qw],
                        start=True, stop=True, tile_position=(r, 0))
                    evac(kw_sb[hh][:, q0:q0 + qw], p[:, :qw], bk_sb[hh])

            def proj_vt():
                # one [128, 64] chunk per rotating PSUM slot (matmul PSUM
                # dst must sit at a bank-aligned offset), evacuated per
                # chunk on the vector engine.
                for kc in range(KC):
                    r = half()
                    p = proj_slot()
                    nc.tensor.matmul(
                        p[:, 0:64],
                        lhsT=x2[r:r + 64, 128 * kc:128 * kc + 128],
                        rhs=wv_sb[r:r + 64, 0:64],
                        start=True, stop=True, tile_position=(r, 0))
                    nc.vector.tensor_tensor(
                        out=vt_bf[:, 64 * kc:64 * kc + 64],
                        in0=p[:, 0:64],
                        in1=bv_sb[:, 0:64],
                        op=ALU.add)

            # --------------- Steady state helpers ---------------------
            _g = [0]
            exp_tiles = {}

            def logits(kc, hh):
                lg = ptile("lg", 1024)
                lgb = ptile("lgb", 1536)
                expst = expp.tile([128, L], BF16, tag="expst", name="expst",
                                  bufs=6)
                sums = small.tile([128, 2], F32, tag="sums", name="sums")
                exp_tiles[(kc, hh)] = (lg, lgb, expst, sums)
                for (q0, qw) in MT:
                    dst = lg[:, q0:q0 + qw] if q0 < 1024 else \
                        lgb[:, q0 - 1024:q0 - 1024 + qw]
                    g = _g[0] % 4
                    _g[0] += 1
                    nc.tensor.matmul(
                        dst,
                        lhsT=kw_sb[hh][32 * g:32 * g + 32,
                                       128 * kc:128 * kc + 128],
                        rhs=qrep_sb[hh][32 * g:32 * g + 32, q0:q0 + qw],
                        start=True, stop=True,
                        tile_position=(32 * g, 0))

            def exp_part(kc, hh, et):
                lg, lgb, expst, sums = exp_tiles[(kc, hh)]
                e0, ew = ET[et]
                src = lg[:, 0:1024] if et == 0 else lgb[:, 0:1280]
                nc.scalar.activation(
                    out=expst[:, e0:e0 + ew],
                    in_=src,
                    func=AF.Exp,
                    accum_out=sums[:, et:et + 1])

            def softmax_o(kc, hh, oa, ob):
                _, _, expst, sums = exp_tiles.pop((kc, hh))
                ssum = small.tile([128, 1], F32, tag="ssum", name="ssum")
                nc.vector.reduce_sum(ssum, sums[:, 0:2],
                                     axis=mybir.AxisListType.X)
                recip = small.tile([128, 1], F32, tag="recip", name="recip")
                nc.vector.reciprocal_approx_fast(recip, ssum)
                vts = small.tile([128, 32], BF16, tag="vts", name="vts")
                nc.vector.tensor_scalar_mul(
                    out=vts,
                    in0=vt_bf[:, 64 * kc + 32 * hh:64 * kc + 32 * hh + 32],
                    scalar1=recip)
                last = (kc, hh) == stripes[-1]
                first = False
                # oa rows 0:64 hold q[0:1024] (h0 0:32 | h1 32:64), rows
                # 64:128 hold q[1024:2048]; ob rows 0:64 hold the tail.
                # Emission alternates row groups so consecutive o matmuls
                # occupy different PE column groups and pair.
                lo, hi = 32 * hh, 64 + 32 * hh
                omap = [(0, oa[lo:lo + 32, 0:512], lo),
                        (2, oa[hi:hi + 32, 0:512], hi),
                        (1, oa[lo:lo + 32, 512:1024], lo),
                        (3, oa[hi:hi + 32, 512:1024], hi),
                        (4, ob[lo:lo + 32, 0:256], lo)]
                for t, dst, col in omap:
                    q0, qw = MT[t]
                    nc.tensor.matmul(
                        dst,
                        lhsT=vts,
                        rhs=expst[:, q0:q0 + qw],
                        start=first, stop=last,
                        tile_position=(0, col),
                        skip_group_check=True)

            # --------------- Emission ---------------------------------
            stripes = [(kc, 0) for kc in range(KC)] + \
                [(kc, 1) for kc in range(KC)]
            proj_qk(0, ["act", "dve"])
            logits(*stripes[0])
            exp_part(*stripes[0], 0)
            proj_qk(1, ["dve"])
            logits(*stripes[1])
            exp_part(*stripes[0], 1)
            logits(*stripes[2])
            proj_vt()

            # persistent o accumulators (created after prologue borrowed
            # the oa/ob tags as scratch); zero-warmed so the o matmuls can
            # always accumulate with start=False
            zero_sb = planes.tile([1, 512], BF16, tag="zero", name="zero")
            nc.vector.memset(zero_sb, 0.0)
            oa = ptile("oa", 1024)
            ob = ptile("ob", 512)
            for dst in (oa[:, 0:512], oa[:, 512:1024], ob[:, 0:512]):
                nc.tensor.matmul(
                    dst,
                    lhsT=zero_sb[:, 0:128],
                    rhs=zero_sb[:, :],
                    start=True, stop=False, skip_group_check=True)

            emitted = 3          # stripes with logits emitted
            exped = [1, 0]       # (fully-exped stripe count, parts of next)

            def pump_exp():
                i, p = exped
                if i >= len(stripes):
                    return
                exp_part(*stripes[i], p)
                if p == 1:
                    exped[0], exped[1] = i + 1, 0
                else:
                    exped[1] = 1

            for i, (kc, hh) in enumerate(stripes):
                while emitted < min(i + 3, len(stripes)):
                    logits(*stripes[emitted])
                    emitted += 1
                # keep ACT one full stripe ahead of the o stage
                while exped[0] < min(i + 2, len(stripes)):
                    pump_exp()
                softmax_o(kc, hh, oa, ob)

            # --------------- Epilogue ---------------------------------
            # q output (bf16, from the replicated projections)
            for hh in range(HPC):
                nc.sync.dma_start(out=q_out_d[8 * hh:8 * hh + 8, :],
                                  in_=qrep_sb[hh][0:8, :])

            # evacuate o: column-split so ACT and DVE run in parallel
            # (disjoint column ranges of o_sb avoid serialization)
            nc.scalar.copy(out=o_sb[:, 0:512], in_=oa[:, 0:512])
            nc.vector.tensor_copy(out=o_sb[:, 512:1024],
                                  in_=oa[:, 512:1024])
            nc.vector.tensor_copy(out=o_sb[:, 1024:1280],
                                  in_=ob[:, 0:256])

            small.release()
            expp.release()
            mp.release()

            # ---- Final projection (bf16) ----
            fp = tc.alloc_tile_pool(name="fpsum", bufs=4, space="PSUM")
            # final 1x1 projection: order alternates PE row halves so
            # consecutive matmuls pair; bias rides the PSUM evacuation
            fmap = [(o_sb[0:64, 0:512], wo_sb[0:64, :], (0, 0)),
                    (o_sb[0:64, 512:1024], wo_sb[0:64, :], (0, 0)),
                    (o_sb[64:128, 0:512], wo_sb[64:128, :], (64, 0)),
                    (o_sb[64:128, 512:1024], wo_sb[64:128, :], (64, 0)),
                    (o_sb[0:64, 1024:1280], wo_sb[0:64, :], (0, 0))]
            for j, t in enumerate((0, 2, 1, 3, 4)):
                q0, qw = MT[t]
                op = fp.tile([64, 512], F32, tag="fo", name="op")
                rhs, lhsT, tp = fmap[t]
                nc.tensor.matmul(op[:, :qw], lhsT=lhsT, rhs=rhs,
                                 start=True, stop=True, tile_position=tp)
                if j % 2 == 0:
                    nc.scalar.add(out_bf[:, q0:q0 + qw], op[0:64, :qw],
                                  bo_sb[0:64, :])
                else:
                    nc.vector.tensor_scalar_add(
                        out=out_bf[:, q0:q0 + qw],
                        in0=op[0:64, :qw],
                        scalar1=bo_sb[0:64, :])
            fp.release()

            nc.sync.dma_start(out=out_d[:, 0:1024], in_=out_bf[:, 0:1024])
            nc.sync.dma_start(out=out_d[:, 1024:L], in_=out_bf[:, 1024:L])

    nc.compile()
    return nc


def make_core_inputs(core, x, Wq, bq, Wk, bk, Wv, bv, Wo, bo):
    b = core // 4
    base = 16 * (core % 4)
    scale = np.float32(DH ** -0.5)

    x_flat = np.asarray(x[b]).reshape(C, L).astype(np.float32)
    x2 = np.zeros((128, L), np.float32)
    x2[0:64] = x_flat
    x2[64:128] = x_flat

    wpb32 = np.zeros((128, WPB_COLS), np.float32)
    wpf = np.zeros((128, WPF_COLS), np.float32)
    for hh in range(HPC):
        ch = slice(base + 8 * hh, base + 8 * hh + 8)
        for half in (0, 64):
            rs = slice(half, half + 64)
            for g in range(4):
                cols = 128 * hh + 32 * g
                wpb32[rs, cols:cols + 8] = Wk[ch].T                # wk_rep
                wpb32[rs, 256 + cols:256 + cols + 8] = (Wq[ch] * scale).T
            wpb32[rs, 512 + 32 * hh:512 + 32 * hh + 8] = Wv[ch].T  # wv_pad
        for g in range(4):
            wpf[32 * g:32 * g + 8, 1 + hh] = bq[ch] * scale
            wpf[32 * g:32 * g + 8, 3 + hh] = bk[ch]
        for rep in range(8):
            c0 = 5 + 64 * rep + 32 * hh
            wpf[:, c0:c0 + 8] = bv[ch][None, :]                    # bv_rep
        # wo_rep, both partition halves
        wpb32[32 * hh:32 * hh + 8, 576:640] = Wo[:, ch].T
        wpb32[64 + 32 * hh:64 + 32 * hh + 8, 576:640] = Wo[:, ch].T
    wpf[0:64, 0] = bo / 4.0

    return dict(x=x2.astype(ml_dtypes.bfloat16),
                wpack_bf=wpb32.astype(ml_dtypes.bfloat16),
                wpack_f32=wpf)


def assemble_outputs(results):
    out_full = np.zeros((B, 64, L), np.float32)
    q_full = np.zeros((B, 64, L), np.float32)
    for core in range(NCORES):
        b = core // 4
        base = 16 * (core % 4)
        q_full[b, base:base + 16] = \
            np.asarray(results[core]["q_part"]).astype(np.float32)
        out_full[b] += np.asarray(results[core]["out_part"]).astype(np.float32)
    return (out_full.reshape(B, 64, HH, WW), q_full.reshape(B, 64, HH, WW))


_NC_CACHE = {}


def get_nc():
    if "nc" not in _NC_CACHE:
        _NC_CACHE["nc"] = build_nc()
    return _NC_CACHE["nc"]


def kernel(**inputs):
    inputs = {k: np.asarray(v) for k, v in inputs.items()}
    nc = get_nc()
    in_maps = [make_core_inputs(c, **inputs) for c in range(NCORES)]
    res = run_bass_kernel_spmd(nc, in_maps, core_ids=list(range(NCORES)))
    return assemble_outputs(res.results)


if __name__ == "__main__":
    import reference
    inputs = {k: np.asarray(v) for k, v in reference.setup_inputs().items()}
    out, q = kernel(**inputs)
    ref_out, ref_q = [np.asarray(v) for v in reference.reference(**inputs)]
    for name, got, want in [("out", out, ref_out), ("q", q, ref_q)]:
        err = np.abs(got - want).max() / np.abs(want).max()
        print(f"{name}: absmax-rel err = {err:.3e}")


# revision 28
# speedup vs baseline: 1.2544x; 1.0030x over previous
"""Trainium2 Bass kernel for nn_Attention_gvtn (8-head spatial attention,
softmax over the query axis), distributed over 8 NeuronCores.

Sharding: data + head parallel. 16 (batch, head) pairs -> 2 heads per core
(same batch). Per core: q/k/v projections for its heads (x duplicated into
both partition halves so alternate projection matmuls occupy disjoint PE
row groups and pair 2-way; biases ride the PSUM evacuation instructions),
the [L, L] logits^T (k on partitions, q on free) via PE row-tiling, exp on
the scalar engine (softmax over q == free axis), per-k sums via a DVE
tensor_scalar pass with the per-lane fp32 accumulator, softmax normalizer
folded into v, o = v' @ exp accumulated in PSUM over key chunks. The final
1x1 output projection produces a per-core bf16 partial; the host sums the
4 partials per batch. The graded q output is returned bf16 directly from
the projected q tiles (well inside the 2e-2 gate).

Self-contained: shapes/sharding hardcoded for x[2,64,48,48], 8 heads.
"""

import os

import numpy as np
import ml_dtypes

import concourse.bacc as bacc
import concourse.bass as bass
import concourse.mybir as mybir
import concourse.tile as tile
from concourse.bass_utils import run_bass_kernel_spmd

F32 = mybir.dt.float32
BF16 = mybir.dt.bfloat16
AF = mybir.ActivationFunctionType
ALU = mybir.AluOpType

B, C, HH, WW = 2, 64, 48, 48
L = HH * WW                   # 2304
NH, DH = 8, 8
NCORES = 8
HPC = 2                       # heads per core
KC = 18                       # key chunks of 128

# Matmul free-dim tiling of the q axis (512-col PSUM bank limit)
MT = [(0, 512), (512, 512), (1024, 512), (1536, 512), (2048, 256)]
# exp/ACT tiling: lg [128,1024] then lgb [128,1280]
ET = [(0, 1024), (1024, 1280)]

# bf16 pack columns: wk_rep 0:256 | wq_rep 256:512 | wv_pad 512:576 |
#                    wo_rep 576:640   (all replicated in both row halves)
WPB_COLS = 640
# f32 pack columns: bo/4 0:1 | bias_q h0,h1 1:3 | bias_k h0,h1 3:5 |
#                   bv_rep x8 5:517
WPF_COLS = 517


def build_nc():
    nc = bacc.Bacc(num_devices=NCORES)

    x_d = nc.declare_dram_parameter("x", [128, L], BF16, isOutput=False)
    wpb_d = nc.declare_dram_parameter("wpack_bf", [128, WPB_COLS], BF16,
                                      isOutput=False)
    wpf_d = nc.declare_dram_parameter("wpack_f32", [128, WPF_COLS], F32,
                                      isOutput=False)

    q_out_d = nc.declare_dram_parameter("q_part", [16, L], BF16, isOutput=True)
    out_d = nc.declare_dram_parameter("out_part", [64, L], BF16, isOutput=True)

    with tile.TileContext(nc, num_cores=NCORES) as tc:
        with tc.tile_pool(name="const", bufs=1) as const_pool, \
             tc.tile_pool(name="planes", bufs=1) as planes:
            wpb = const_pool.tile([128, WPB_COLS], BF16)
            nc.sync.dma_start(out=wpb, in_=wpb_d[:, :])
            x2 = const_pool.tile([128, L], BF16)
            nc.sync.dma_start(out=x2[:, 0:512], in_=x_d[:, 0:512])
            wpf = const_pool.tile([128, WPF_COLS], F32)
            nc.sync.dma_start(out=wpf, in_=wpf_d[:, :])
            for c0 in range(512, L, 640):
                cw = min(640, L - c0)
                nc.sync.dma_start(out=x2[:, c0:c0 + cw],
                                  in_=x_d[:, c0:c0 + cw])

            wk_sb = wpb[:, 0:256]
            wq_sb = wpb[:, 256:512]
            wv_sb = wpb[:, 512:576]
            wo_sb = wpb[:, 576:640]
            bo_sb = wpf[:, 0:1]          # rows 0:64 = bo/4
            bq_sb = [wpf[:, 1 + h:2 + h] for h in range(HPC)]
            bk_sb = [wpf[:, 3 + h:4 + h] for h in range(HPC)]
            bv_sb = wpf[:, 5:517]        # [128, 512] bv pattern repeated x8

            qrep_sb, kw_sb = [], []
            for hh in range(HPC):
                qrep_sb.append(planes.tile([128, L], BF16, tag=f"qrep{hh}",
                                           name=f"qrep{hh}"))
                kw_sb.append(planes.tile([128, L], BF16, tag=f"kw{hh}",
                                         name=f"kw{hh}"))
            vt_bf = planes.tile([128, KC * 64], BF16, tag="vtbf", name="vtbf")
            o_sb = planes.tile([128, 1280], BF16, tag="osb", name="osb")
            out_bf = planes.tile([64, L], BF16, tag="outbf", name="outbf")
            scratch = planes.tile([128, L], BF16, tag="scr", name="scr")

            # ---- PSUM: one pool, tags: lg(2) lgb(3) oa(2) ob(1) = 8 banks
            mp = tc.alloc_tile_pool(name="mainpsum", bufs=1, space="PSUM")
            expp = tc.alloc_tile_pool(name="expp", bufs=6)
            small = tc.alloc_tile_pool(name="small", bufs=6)

            def ptile(tag, cols=1024):
                return mp.tile([128, cols], F32, tag=tag, name=tag, bufs=1)

            # --------------- Prologue: projections --------------------
            # scratch arenas over the oa/ob banks only, so lg/lgb stay
            # free for the first logits stripes. 3 rotating 512-col slots.
            arena_a = ptile("oa", 1024)
            arena_b = ptile("ob", 512)
            _slots = [arena_a[:, 0:512], arena_a[:, 512:1024],
                      arena_b[:, 0:512]]
            _rot = [0]

            def proj_slot(width=512):
                s = _slots[_rot[0] % 3]
                _rot[0] += 1
                return s

            _half = [0]

            def half():
                # alternate PE row halves so consecutive projections pair
                h = _half[0]
                _half[0] ^= 1
                return h * 64

            def proj_qk(hh, engines):
                ei = [0]

                def evac(dst, src, bias):
                    e = engines[ei[0] % len(engines)]
                    ei[0] += 1
                    if e == "act":
                        nc.scalar.add(dst, src, bias)
                    else:
                        nc.vector.tensor_scalar_add(out=dst, in0=src,
                                                    scalar1=bias)

                for (q0, qw) in MT:
                    r = half()
                    p = proj_slot()
                    nc.tensor.matmul(
                        p[:, :qw],
                        lhsT=wq_sb[r:r + 64, 128 * hh:128 * hh + 128],
                        rhs=x2[r:r + 64, q0:q0 + qw],
                        start=True, stop=True, tile_position=(r, 0))
                    evac(qrep_sb[hh][:, q0:q0 + qw], p[:, :qw], bq_sb[hh])
                    r = half()
                    p = proj_slot()
                    nc.tensor.matmul(
                        p[:, :qw],
                        lhsT=wk_sb[r:r + 64, 128 * hh:128 * hh + 128],
                        rhs=x2[r:r + 64, q0:q0 + qw],
                        start=True, stop=True, tile_position=(r, 0))
                    evac(kw_sb[hh][:, q0:q0 + qw], p[:, :qw], bk_sb[hh])

            def proj_vt():
                # one [128, 64] chunk per rotating PSUM slot (matmul PSUM
                # dst must sit at a bank-aligned offset), evacuated per
                # chunk on the vector engine.
                for kc in range(KC):
                    r = half()
                    p = proj_slot()
                    nc.tensor.matmul(
                        p[:, 0:64],
                        lhsT=x2[r:r + 64, 128 * kc:128 * kc + 128],
                        rhs=wv_sb[r:r + 64, 0:64],
                        start=True, stop=True, tile_position=(r, 0))
                    nc.vector.tensor_tensor(
                        out=vt_bf[:, 64 * kc:64 * kc + 64],
                        in0=p[:, 0:64],
                        in1=bv_sb[:, 0:64],
                        op=ALU.add)

            # --------------- Steady state helpers ---------------------
            _g = [0]
            exp_tiles = {}

            def logits(kc, hh):
                lg = ptile("lg", 1024)
                lgb = ptile("lgb", 1536)
                expst = expp.tile([128, L], BF16, tag="expst", name="expst",
                                  bufs=6)
                sums = small.tile([128, 2], F32, tag="sums", name="sums")
                exp_tiles[(kc, hh)] = (lg, lgb, expst, sums)
                for (q0, qw) in MT:
                    dst = lg[:, q0:q0 + qw] if q0 < 1024 else \
                        lgb[:, q0 - 1024:q0 - 1024 + qw]
                    g = _g[0] % 4
                    _g[0] += 1
                    nc.tensor.matmul(
                        dst,
                        lhsT=kw_sb[hh][32 * g:32 * g + 32,
                                       128 * kc:128 * kc + 128],
                        rhs=qrep_sb[hh][32 * g:32 * g + 32, q0:q0 + qw],
                        start=True, stop=True,
                        tile_position=(32 * g, 0))

            def exp_part(kc, hh, et):
                lg, lgb, expst, sums = exp_tiles[(kc, hh)]
                e0, ew = ET[et]
                src = lg[:, 0:1024] if et == 0 else lgb[:, 0:1280]
                nc.scalar.activation(
                    out=expst[:, e0:e0 + ew],
                    in_=src,
                    func=AF.Exp,
                    accum_out=sums[:, et:et + 1])

            def softmax_o(kc, hh, oa, ob):
                _, _, expst, sums = exp_tiles.pop((kc, hh))
                ssum = small.tile([128, 1], F32, tag="ssum", name="ssum")
                nc.vector.reduce_sum(ssum, sums[:, 0:2],
                                     axis=mybir.AxisListType.X)
                recip = small.tile([128, 1], F32, tag="recip", name="recip")
                nc.vector.reciprocal_approx_fast(recip, ssum)
                vts = small.tile([128, 32], BF16, tag="vts", name="vts")
                nc.vector.tensor_scalar_mul(
                    out=vts,
                    in0=vt_bf[:, 64 * kc + 32 * hh:64 * kc + 32 * hh + 32],
                    scalar1=recip)
                last = (kc, hh) == stripes[-1]
                first = False
                # oa rows 0:64 hold q[0:1024] (h0 0:32 | h1 32:64), rows
                # 64:128 hold q[1024:2048]; ob rows 0:64 hold the tail.
                # Emission alternates row groups so consecutive o matmuls
                # occupy different PE column groups and pair.
                lo, hi = 32 * hh, 64 + 32 * hh
                omap = [(0, oa[lo:lo + 32, 0:512], lo),
                        (2, oa[hi:hi + 32, 0:512], hi),
                        (1, oa[lo:lo + 32, 512:1024], lo),
                        (3, oa[hi:hi + 32, 512:1024], hi),
                        (4, ob[lo:lo + 32, 0:256], lo)]
                for t, dst, col in omap:
                    q0, qw = MT[t]
                    nc.tensor.matmul(
                        dst,
                        lhsT=vts,
                        rhs=expst[:, q0:q0 + qw],
                        start=first, stop=last,
                        tile_position=(0, col),
                        skip_group_check=True)

            # --------------- Emission ---------------------------------
            # first two stripes are head-0 so their logits never stall
            # on the head-1 evacuations
            stripes = [(0, 0), (1, 0), (0, 1), (1, 1)] + \
                [(kc, hh) for kc in range(2, KC) for hh in range(HPC)]
            proj_qk(0, ["act", "dve"])
            logits(*stripes[0])
            exp_part(*stripes[0], 0)
            exp_part(*stripes[0], 1)
            proj_qk(1, ["act", "dve", "dve", "dve"])
            logits(*stripes[1])
            proj_vt()

            # persistent o accumulators (created after prologue borrowed
            # the oa/ob tags as scratch); zero-warmed so the o matmuls can
            # always accumulate with start=False
            zero_sb = planes.tile([1, 512], BF16, tag="zero", name="zero")
            nc.vector.memset(zero_sb, 0.0)
            oa = ptile("oa", 1024)
            ob = ptile("ob", 512)
            for dst in (oa[:, 0:512], oa[:, 512:1024], ob[:, 0:512]):
                nc.tensor.matmul(
                    dst,
                    lhsT=zero_sb[:, 0:128],
                    rhs=zero_sb[:, :],
                    start=True, stop=False, skip_group_check=True)

            emitted = 2          # stripes with logits emitted
            exped = [1, 0]       # (fully-exped stripe count, parts of next)

            def pump_exp():
                i, p = exped
                if i >= len(stripes):
                    return
                exp_part(*stripes[i], p)
                if p == 1:
                    exped[0], exped[1] = i + 1, 0
                else:
                    exped[1] = 1

            for i, (kc, hh) in enumerate(stripes):
                while emitted < min(i + 3, len(stripes)):
                    logits(*stripes[emitted])
                    emitted += 1
                # keep ACT one full stripe ahead of the o stage
                while exped[0] < min(i + 2, len(stripes)):
                    pump_exp()
                softmax_o(kc, hh, oa, ob)

            # --------------- Epilogue ---------------------------------
            # q output (bf16, from the replicated projections)
            for hh in range(HPC):
                nc.sync.dma_start(out=q_out_d[8 * hh:8 * hh + 8, :],
                                  in_=qrep_sb[hh][0:8, :])

            # evacuate o: column-split so ACT and DVE run in parallel
            # (disjoint column ranges of o_sb avoid serialization)
            nc.scalar.copy(out=o_sb[:, 0:512], in_=oa[:, 0:512])
            nc.vector.tensor_copy(out=o_sb[:, 512:1024],
                                  in_=oa[:, 512:1024])
            nc.vector.tensor_copy(out=o_sb[:, 1024:1280],
                                  in_=ob[:, 0:256])

            small.release()
            expp.release()
            mp.release()

            # ---- Final projection (bf16) ----
            fp = tc.alloc_tile_pool(name="fpsum", bufs=4, space="PSUM")
            # final 1x1 projection: order alternates PE row halves so
            # consecutive matmuls pair; bias rides the PSUM evacuation
            fmap = [(o_sb[0:64, 0:512], wo_sb[0:64, :], (0, 0)),
                    (o_sb[0:64, 512:1024], wo_sb[0:64, :], (0, 0)),
                    (o_sb[64:128, 0:512], wo_sb[64:128, :], (64, 0)),
                    (o_sb[64:128, 512:1024], wo_sb[64:128, :], (64, 0)),
                    (o_sb[0:64, 1024:1280], wo_sb[0:64, :], (0, 0))]
            for j, t in enumerate((0, 2, 1, 3, 4)):
                q0, qw = MT[t]
                op = fp.tile([64, 512], F32, tag="fo", name="op")
                rhs, lhsT, tp = fmap[t]
                nc.tensor.matmul(op[:, :qw], lhsT=lhsT, rhs=rhs,
                                 start=True, stop=True, tile_position=tp)
                if j % 2 == 0:
                    nc.scalar.add(out_bf[:, q0:q0 + qw], op[0:64, :qw],
                                  bo_sb[0:64, :])
                else:
                    nc.vector.tensor_scalar_add(
                        out=out_bf[:, q0:q0 + qw],
                        in0=op[0:64, :qw],
                        scalar1=bo_sb[0:64, :])
            fp.release()

            nc.sync.dma_start(out=out_d[:, 0:1024], in_=out_bf[:, 0:1024])
            nc.sync.dma_start(out=out_d[:, 1024:L], in_=out_bf[:, 1024:L])

    nc.compile()
    return nc


def make_core_inputs(core, x, Wq, bq, Wk, bk, Wv, bv, Wo, bo):
    b = core // 4
    base = 16 * (core % 4)
    scale = np.float32(DH ** -0.5)

    x_flat = np.asarray(x[b]).reshape(C, L).astype(np.float32)
    x2 = np.zeros((128, L), np.float32)
    x2[0:64] = x_flat
    x2[64:128] = x_flat

    wpb32 = np.zeros((128, WPB_COLS), np.float32)
    wpf = np.zeros((128, WPF_COLS), np.float32)
    for hh in range(HPC):
        ch = slice(base + 8 * hh, base + 8 * hh + 8)
        for half in (0, 64):
            rs = slice(half, half + 64)
            for g in range(4):
                cols = 128 * hh + 32 * g
                wpb32[rs, cols:cols + 8] = Wk[ch].T                # wk_rep
                wpb32[rs, 256 + cols:256 + cols + 8] = (Wq[ch] * scale).T
            wpb32[rs, 512 + 32 * hh:512 + 32 * hh + 8] = Wv[ch].T  # wv_pad
        for g in range(4):
            wpf[32 * g:32 * g + 8, 1 + hh] = bq[ch] * scale
            wpf[32 * g:32 * g + 8, 3 + hh] = bk[ch]
        for rep in range(8):
            c0 = 5 + 64 * rep + 32 * hh
            wpf[:, c0:c0 + 8] = bv[ch][None, :]                    # bv_rep
        # wo_rep, both partition halves
        wpb32[32 * hh:32 * hh + 8, 576:640] = Wo[:, ch].T
        wpb32[64 + 32 * hh:64 + 32 * hh + 8, 576:640] = Wo[:, ch].T
    wpf[0:64, 0] = bo / 4.0

    return dict(x=x2.astype(ml_dtypes.bfloat16),
                wpack_bf=wpb32.astype(ml_dtypes.bfloat16),
                wpack_f32=wpf)


def assemble_outputs(results):
    out_full = np.zeros((B, 64, L), np.float32)
    q_full = np.zeros((B, 64, L), np.float32)
    for core in range(NCORES):
        b = core // 4
        base = 16 * (core % 4)
        q_full[b, base:base + 16] = \
            np.asarray(results[core]["q_part"]).astype(np.float32)
        out_full[b] += np.asarray(results[core]["out_part"]).astype(np.float32)
    return (out_full.reshape(B, 64, HH, WW), q_full.reshape(B, 64, HH, WW))


_NC_CACHE = {}


def get_nc():
    if "nc" not in _NC_CACHE:
        _NC_CACHE["nc"] = build_nc()
    return _NC_CACHE["nc"]


def kernel(**inputs):
    inputs = {k: np.asarray(v) for k, v in inputs.items()}
    nc = get_nc()
    in_maps = [make_core_inputs(c, **inputs) for c in range(NCORES)]
    res = run_bass_kernel_spmd(nc, in_maps, core_ids=list(range(NCORES)))
    return assemble_outputs(res.results)


if __name__ == "__main__":
    import reference
    inputs = {k: np.asarray(v) for k, v in reference.setup_inputs().items()}
    out, q = kernel(**inputs)
    ref_out, ref_q = [np.asarray(v) for v in reference.reference(**inputs)]
    for name, got, want in [("out", out, ref_out), ("q", q, ref_q)]:
        err = np.abs(got - want).max() / np.abs(want).max()
        print(f"{name}: absmax-rel err = {err:.3e}")
